# revision 37
# baseline (speedup 1.0000x reference)
"""Batch-parallel dot-product attention for Trainium2 (Bass/Tile), v2.

Problem: B=8, Q=K=2048, D=128, fp32, with a [B, K] 0/1 attention mask.
Sharding: one batch element per NeuronCore (8 cores), no collectives.

Architecture (query-subblock-major):

The host compacts the context per batch: kept keys (mask=1) are gathered
on the host into a padded [nctx, D] context (nctx = roundup(max kept)),
with zero rows as padding.  K^T ships pre-transposed in bf16; V ships as
fp16 rows with the key-validity mask appended as a 129th column
([V | mk]); Q^T ships in bf16.  All mask handling is data: padding keys
have zero V rows and mk=0, so they contribute exactly nothing to the
numerator (E x 0) or denominator (E x mk sums), and no bias add or
on-device gather/transpose is needed at all.  An all-masked batch is the
reference's uniform softmax; the host sends qt=0 and mk=1 for all keys,
making E identically 1 (exact).

Device: the 16x9 (qs, k-tile) score tiles form one flat sequence,
packed into PSUM slots of up to 12 tiles (3 banks); E lives in one flat
SBUF tensor so a single exp instruction can cross qs boundaries:
  1. Phase A: per slot tile, S[k_tile, q128] = kT_tile.T @ qT_qs
     (bf16, N=128) into the slot at tile granularity.
  2. One exp per slot (FD up to 1536) on ScalarE, PSUM -> SBUF fp16,
     scale=1/sqrt(D) folded in.  ScalarE is the bottleneck engine; the
     14 slot-exps (sizes [5, 12x11, 5, 2]) run gap-free back to back;
     the small first/last slots shorten pipeline fill and drain.
  3. Phase B: 9 matmuls accumulating O[q128, 129] = sum_t E_t.T @ [V|mk]_t
     in a 1-bank PSUM accumulator; the mk column makes the softmax
     denominator a free by-product (and implements the mask).
  4. Normalize on VectorE (reciprocal + per-partition scalar multiply)
     as each subblock's accumulation completes; stores alternate between
     the HWDGE and Pool-SWDGE descriptor-gen tracks.

Two PSUM score slots (3 banks each) + two accumulators (1 bank each)
fill all 8 banks and give a 2-deep software pipeline: A'(qs+2) and
B(qs) run on the PE while exp(qs+1) runs on ScalarE, keeping the exp
stream gap-free.  Dummy fp16 matmuls warm the PE clock gate and a dummy
exp front-loads the ACT table load during the input DMA wait.

Startup is DMA-latency-bound: descriptor generation (HWDGE) is a serial
track shared by the sync/scalar queues and transfers serialize on the
DMA engines, so exactly what exp(0) needs ships as ONE minimal leading
DMA (hdr = [Q^T cols 0:256 | K^T tiles 0:5], host-packed — the first
exp slot's inputs); the remaining K^T tiles follow first on the scalar
ring (A'(1) needs them only by exp(0)-end), then mid Q^T, [V|mk], and
the Q^T tail.  Only the last subblock's store chain (~2.5us of fixed
DMA issue/transfer/semaphore latency) plus the drain epilogue trail
the exp stream.
"""

import math
from contextlib import ExitStack

import numpy as np

import concourse.bass as bass
import concourse.mybir as mybir
import concourse.tile as tile
from concourse import bacc
from concourse.bass import ds, ts

B = 8
SEQ = 2048
D = 128
P = 128
VROW = 132  # fp16 row: [V (128) | mk (1) | pad (3)]
HDR0 = 256  # leading Q^T columns packed into the hdr tensor
KTH = 5     # K^T tiles packed into the hdr tensor (the first exp slot)

F32 = mybir.dt.float32
BF16 = mybir.dt.bfloat16
F16 = mybir.dt.float16

NWARM = 21  # PE clock-gate warm matmuls before the first real work
NWARM_FINE = 8  # small trailing warm matmuls (fine-grained busy-keeping)


def attention_kernel(tc, qt, hdr, kth, vp, o, seq, nctx):
    nc = tc.nc
    nkt = nctx // P
    nqs = seq // P
    scale = 1.0 / math.sqrt(D)
    exp_f = mybir.ActivationFunctionType.Exp

    # The 144 (qs, k-tile) score tiles are processed as a flat sequence,
    # packed into PSUM slots of up to 12 tiles (3 banks) each — exp
    # instructions deliberately cross qs boundaries (et is one flat SBUF
    # tensor) so the per-instruction ACT overhead amortizes over FD=1536.
    # A small first slot starts the exp stream as soon as the leading A'
    # matmuls land; the small last slot leaves only a few B matmuls
    # trailing the stream.
    total = nqs * nkt
    if total <= 5:
        sizes = [total]
    else:
        # first slot 5 tiles: exp(0) starts as soon as 5 A' matmuls land
        # while A'(1) still prefills under exp(0); last slots (5, 2) so
        # only two B matmuls trail the exp stream before the final
        # normalize/store chain.
        sizes = [5]
        rem = total - 7
        while rem > 0:
            s = min(12, rem)
            sizes.append(s)
            rem -= s
        sizes.append(2)
    gstarts = [sum(sizes[:i]) for i in range(len(sizes))]
    cap = max(sizes)
    units = list(range(len(sizes)))

    with ExitStack() as ctx:
        sb = ctx.enter_context(tc.tile_pool(name="sb", bufs=1))
        # 6 output-staging buffers: the last subblocks normalize in a burst
        # at stream end, and the staging tile must not wait on an earlier
        # store's DMA-completion semaphore (+900ns each)
        obp = ctx.enter_context(tc.tile_pool(name="obp", bufs=6))
        smallp = ctx.enter_context(tc.tile_pool(name="smallp", bufs=4))
        psS = ctx.enter_context(tc.tile_pool(name="psS", bufs=2, space="PSUM"))
        psO = ctx.enter_context(tc.tile_pool(name="psO", bufs=2, space="PSUM"))

        # HWDGE descriptor generation (625ns/DMA) is a serial track shared
        # by the sync and scalar queues, and transfers serialize on the DMA
        # engines — so the critical path wants ONE minimal leading DMA
        # carrying exactly what exp(0) needs: hdr = [Q^T cols 0:256 | K^T
        # tiles 0:KTH] (host-packed, = the first exp slot's inputs).
        kt0 = min(KTH * P, nctx)
        hdrt = sb.tile([P, HDR0 + kt0], BF16)
        nc.sync.dma_start(hdrt, hdr)
        qc0 = hdrt[:, 0:HDR0]
        if nctx > kt0:
            ktht = sb.tile([P, nctx - kt0], BF16)

        def kt_tile(t):
            if (t + 1) * P <= kt0:
                return hdrt[:, HDR0 + t * P : HDR0 + (t + 1) * P]
            return ktht[:, ts(t - kt0 // P, P)]

        # scalar ring: the remaining K^T tiles first (A'(1) needs them by
        # exp(0)-end), then mid Q^T (gates A'(2..)), then [V|mk] (first
        # needed by B(0), well after exp(0)), then the tail of Q^T.
        qtt = sb.tile([P, seq], BF16)
        vpt = sb.tile([P, nkt, VROW], F16)
        if nctx > kt0:
            nc.scalar.dma_start(ktht, kth)
        c1 = min(seq, HDR0 + 8 * P)
        if c1 > HDR0:
            nc.scalar.dma_start(qtt[:, HDR0:c1], qt[:, HDR0:c1])
        nc.scalar.dma_start(vpt.rearrange("p t d -> p (t d)"), vp)
        if c1 < seq:
            nc.scalar.dma_start(qtt[:, c1:seq], qt[:, c1:seq])

        # Dummy exp so walrus front-loads the ACT table load under the DMAs.
        # Memsets go to the otherwise-idle GpSimd engine so the PE warm
        # matmuls (below) start as early as possible — the clock-gate ramp
        # needs 3us of continuous PE busy before full rate.
        warm = smallp.tile([P, 1], F32, tag="warm")
        nc.gpsimd.memset(warm, 0.0)
        nc.scalar.activation(warm, warm, exp_f)

        # PE clock-gate warm: keep the PE busy through the input-DMA wait so
        # the real matmuls start at full rate.
        wm = smallp.tile([P, P], F16, tag="wm")
        nc.gpsimd.memset(wm, 0.0)
        pw = psO.tile([P, VROW], F32, tag="oacc", name="pw")
        for _ in range(NWARM):
            nc.tensor.matmul(pw[:, 0:P], lhsT=wm, rhs=wm, start=True, stop=True)
        # fine-grained warm tail (N=32) so the PE stays busy until the hdr
        # DMA lands without overshooting into A'(0)'s start
        for _ in range(NWARM_FINE):
            nc.tensor.matmul(
                pw[:, 0:32], lhsT=wm, rhs=wm[:, 0:32], start=True, stop=True
            )

        # E for every (qs, k-tile) score tile lives in one flat SBUF tensor
        # so a single exp instruction may span a qs boundary.
        et_all = sb.tile([P, total, P], F16, name="et_all")

        slots = {}
        oaccs = {}

        def emit_A(u):
            sl = psS.tile([P, cap, P], F32, tag="sl", name=f"sl_{u}")
            slots[u] = sl
            for j in range(sizes[u]):
                qs, t = divmod(gstarts[u] + j, nkt)
                rhs = qc0 if qs * P < HDR0 else qtt
                nc.tensor.matmul(
                    sl[:, j, :], lhsT=kt_tile(t), rhs=rhs[:, ts(qs, P)],
                    start=True, stop=True,
                )

        def emit_exp(u):
            g0, sz = gstarts[u], sizes[u]
            sl = slots.pop(u)
            nc.scalar.activation(
                et_all[:, g0 : g0 + sz, :], sl[:, 0:sz, :], exp_f, scale=scale
            )

        def emit_norm_store(qs):
            oa = oaccs.pop(qs)
            r = smallp.tile([P, 1], F32, tag="r")
            nc.vector.reciprocal(r, oa[:, D : D + 1])
            ob = obp.tile([P, P], F32, tag="ob", name=f"ob{qs}")
            nc.vector.tensor_scalar_mul(ob, oa[:, 0:D], r)
            # Stores alternate between the two independent descriptor-gen
            # tracks (HWDGE 625ns vs Pool-SWDGE ~1037ns) so the end-clustered
            # stores don't serialize their gens; the critical last store
            # (odd qs15) rides the faster HWDGE track.
            eng = nc.sync if qs % 2 else nc.gpsimd
            eng.dma_start(o[ds(qs * P, P), :], ob)

        def emit_B(u):
            for j in range(sizes[u]):
                g = gstarts[u] + j
                qs, t = divmod(g, nkt)
                if t == 0:
                    oaccs[qs] = psO.tile(
                        [P, VROW], F32, tag="oacc", name=f"oacc{qs}"
                    )
                nc.tensor.matmul(
                    oaccs[qs][:, 0 : D + 1], lhsT=et_all[:, g, :],
                    rhs=vpt[:, t, 0 : D + 1],
                    start=(t == 0), stop=(t == nkt - 1),
                )
                if t == nkt - 1:
                    emit_norm_store(qs)

        emit_A(units[0])
        if len(units) > 1:
            emit_A(units[1])
        for i, u in enumerate(units):
            emit_exp(u)
            if i + 2 < len(units):
                emit_A(units[i + 2])
            emit_B(u)


def build_nc(seq=SEQ, nctx=SEQ, n_cores=B):
    nkt = nctx // P
    nc = bacc.Bacc(
        "TRN2", target_bir_lowering=False, debug=False, num_devices=n_cores
    )
    kt0 = min(KTH * P, nctx)
    qt = nc.dram_tensor("qt", [D, seq], BF16, kind="ExternalInput").ap()
    hdr = nc.dram_tensor("hdr", [D, HDR0 + kt0], BF16, kind="ExternalInput").ap()
    kth = (
        nc.dram_tensor("kth", [D, nctx - kt0], BF16, kind="ExternalInput").ap()
        if nctx > kt0 else None
    )
    vp = nc.dram_tensor("vp", [P, nkt * VROW], F16, kind="ExternalInput").ap()
    o = nc.dram_tensor("o", [seq, D], F32, kind="ExternalOutput").ap()
    with nc.allow_low_precision("softmax reciprocal on VectorE"):
        with tile.TileContext(nc) as tc:
            attention_kernel(tc, qt, hdr, kth, vp, o, seq, nctx)
    nc.compile()
    return nc


_NC_CACHE = {}


def _get_nc(seq, nctx):
    key = (seq, nctx)
    if key not in _NC_CACHE:
        _NC_CACHE[key] = build_nc(seq=seq, nctx=nctx)
    return _NC_CACHE[key]


def prepare(queries, keys, values, attntion_mask):
    """Host-side layout prep: per-batch compacted context in low precision.

    Returns (nctx, in_maps)."""
    import ml_dtypes

    bf = ml_dtypes.bfloat16
    nb, seq, d = queries.shape
    masks = np.asarray(attntion_mask) != 0
    kept = [np.flatnonzero(masks[b]) for b in range(nb)]
    ns = [int(k.size) for k in kept]
    if min(ns) == 0:
        nctx = seq
    else:
        nctx = min(seq, ((max(ns) + P - 1) // P) * P)
    nkt = nctx // P
    in_maps = []
    for b in range(nb):
        n = ns[b]
        if n == 0:
            # all-masked: reference degenerates to a uniform softmax over
            # every key; qt=0 makes E identically 1, which is exact.
            idx = np.arange(nctx)
            mk = np.ones(nctx, np.float32)
            qtb = np.zeros((d, seq), np.float32)
        else:
            idx = np.zeros(nctx, np.int64)
            idx[:n] = kept[b]
            mk = np.zeros(nctx, np.float32)
            mk[:n] = 1.0
            qtb = queries[b].T
        kc = keys[b][idx] * mk[:, None]
        vc = values[b][idx] * mk[:, None]
        vpa = np.zeros((P, nkt, VROW), np.float16)
        vpa[:, :, 0:d] = vc.reshape(nkt, P, d).transpose(1, 0, 2)
        vpa[:, :, d] = mk.reshape(nkt, P).T
        qtb16 = np.ascontiguousarray(qtb).astype(bf)
        ktb16 = np.ascontiguousarray(kc.T).astype(bf)
        kt0 = min(KTH * P, nctx)
        m = {
            "qt": qtb16,
            "hdr": np.ascontiguousarray(
                np.concatenate([qtb16[:, 0:HDR0], ktb16[:, 0:kt0]], axis=1)
            ),
            "vp": np.ascontiguousarray(vpa.reshape(P, nkt * VROW)),
        }
        if nctx > kt0:
            m["kth"] = np.ascontiguousarray(ktb16[:, kt0:])
        in_maps.append(m)
    return nctx, in_maps


def kernel(queries, keys, values, attntion_mask, **run_kwargs):
    from concourse.bass_utils import run_bass_kernel_spmd

    queries = np.asarray(queries)
    keys = np.asarray(keys)
    values = np.asarray(values)
    attntion_mask = np.asarray(attntion_mask)
    nctx, in_maps = prepare(queries, keys, values, attntion_mask)
    nc = _get_nc(queries.shape[1], nctx)
    res = run_bass_kernel_spmd(
        nc,
        in_maps,
        core_ids=list(range(queries.shape[0])),
        **run_kwargs,
    )
    out = np.stack([r["o"] for r in res.results], axis=0).astype(np.float32)
    if run_kwargs:
        kernel.last_results = res
    return out
